# revision 21
# baseline (speedup 1.0000x reference)
"""AugmentedTripletLoss on 8 TRN2 NeuronCores — data-parallel Bass kernel.

v4 design (data-parallel over batch, 16384 samples/core):
  The O(N*C*D) work — all centroid-to-sample cosine distances, their
  relu thresholds, and the per-class masked reductions — runs on the
  8 cores. The tiny O(N*D) prep (normalization, fp8 quantization,
  class-centroid means, the [16,16] pair mask) and the final [16,17]
  gather-sum over the 8 local partials run on the host, so the device
  program has no collective (measured: the first collective in a NEFF
  pays a ~58us ncfw entry-barrier floor plus ~20us per AllReduce —
  about 45% of the v3 kernel span).

  Device per 512-sample iteration (32 iterations/core):
    16 accumulating matmuls (eT d-chunk stationary fp8, chT moving
    bf16) -> 16*dot in PSUM [128, 4x16];
    one ACT relu -> inter terms (strided into qi[., s, 0:16]);
    DVE masked negate-mult + grouped row-reduce -> own-class dots;
    DVE add+max -> intra terms qi[., s, 16];
    4 one-hot matmuls accumulate S^T ++ intra sums into ps_st [16,17].
  eT streams tile-major from HBM through a 10-deep 256KB chunk pool,
  rotating across the sync/scalar/vector HWDGE queues, so transfers
  overlap each other and the compute trails the stream by one chunk.
"""

import sys

sys.path.insert(0, "/opt/trn_rl_repo")

import numpy as np

import concourse.bass as bass
import concourse.bacc as bacc
import concourse.tile as tile
import concourse.mybir as mybir
from concourse.bass_utils import run_bass_kernel_spmd

ALPHA = 0.1
BETA = 1.1
C = 16
N = 131072
D = 512
CORES = 8
NL = N // CORES  # 16384 samples per core
P = 128
T = NL // P      # 128 sample tiles of 128 per core
B4 = 8           # sample tiles per device iteration
IT = T // B4     # 16 iterations
KCH = D // P     # 4 contraction chunks of 128
ESC = 16.0       # fp8 storage scale for ehat

F32 = mybir.dt.float32
BF16 = mybir.dt.bfloat16
FP8 = mybir.dt.float8e4
ALU = mybir.AluOpType
ACTF = mybir.ActivationFunctionType

_CACHE = {}


def _build():
    nc = bacc.Bacc("TRN2", target_bir_lowering=False, debug=False, num_devices=CORES)

    e8T = nc.dram_tensor("e8T", [P, T * 4 * P], FP8, kind="ExternalInput")
    ohb = nc.dram_tensor("ohb", [P, T * C], BF16, kind="ExternalInput")
    chTin = nc.dram_tensor("chTin", [P, KCH * C], BF16, kind="ExternalInput")
    out = nc.dram_tensor("out", [C, 2 * C], F32, kind="ExternalOutput")

    with tile.TileContext(nc) as tc:
        with (
            tc.tile_pool(name="pers", bufs=1) as pers,
            tc.tile_pool(name="work", bufs=6) as work,
            tc.tile_pool(name="ld", bufs=16) as ld,
            tc.tile_pool(name="small", bufs=1) as small,
            tc.tile_pool(name="psacc", bufs=1, space="PSUM") as psacc,
            tc.tile_pool(name="pstr", bufs=4, space="PSUM") as pstr,
        ):
            # ---- persistent SBUF state ----
            oh_sb = pers.tile([P, T * C], BF16)  # one-hot
            chT = pers.tile([P, KCH * C], BF16)  # d-major normalized centroids
            bq = pers.tile([P, 1], F32)          # 16*(BETA-1) bias

            nc.sync.dma_start(chT[:], chTin[:, :])
            nc.gpsimd.dma_start(oh_sb[:], ohb[:, :])
            nc.vector.memset(bq[:], float(ESC * (BETA - 1.0)))

            # the full 8MB input is SBUF-resident: issue every chunk DMA
            # upfront on the sync+scalar HWDGE queues before any compute
            # op enters those queues (gpsimd/SWDGE is avoided for bulk
            # streaming — the framework's mid-kernel DRAIN blocks it for
            # ~10us)
            HW_ = B4 * KCH * P // 2  # half-iteration chunk: 2048 cols (256KB)
            chunks = []
            for j in range(2 * IT):
                ech = ld.tile([P, HW_], FP8, tag=f"ech{j % 2}")
                eng = nc.sync if j % 2 == 0 else nc.scalar
                eng.dma_start(ech[:], e8T[:, j * HW_:(j + 1) * HW_])
                chunks.append(ech)
            ps_st = psacc.tile([C, 2 * C], F32)

            # the per-iteration one-hot accumulation matmuls are deferred
            # DELAY iterations: the PE queue is in-order, so an acc matmul
            # waiting on iteration i's ACT/DVE output must not sit in
            # front of iteration i+1's dot matmuls
            DELAY = 2
            qis = []

            def emit_acc(i):
                qi_i = qis[i]
                for s in range(B4):
                    t = B4 * i + s
                    nc.tensor.matmul(ps_st[:],
                                     oh_sb[:, t * C:(t + 1) * C],
                                     qi_i[:, s * 2 * C:(s + 1) * 2 * C],
                                     start=(t == 0), stop=(t == T - 1))

            for it in range(IT):
                ech0 = chunks[2 * it]
                ech1 = chunks[2 * it + 1]
                dot = pstr.tile([P, B4 * C], F32, tag="tp")
                for s in range(B4):
                    ech = ech0 if s < B4 // 2 else ech1
                    sl = s % (B4 // 2)
                    for k in range(KCH):
                        c0 = (sl * KCH + k) * P
                        nc.tensor.matmul(
                            dot[:, s * C:(s + 1) * C],
                            ech[:, c0:c0 + P],
                            chT[:, k * C:(k + 1) * C],
                            start=(k == 0), stop=(k == KCH - 1))
                qi = work.tile([P, B4 * 2 * C], BF16)
                qiv = qi[:].rearrange("p (s c) -> p s c", c=2 * C)
                # inter: 16*relu(dot/16 + (BETA-1)) = relu(16dot + 16(B-1))
                nc.scalar.activation(qiv[:, :, 0:C], dot[:], ACTF.Relu,
                                     bias=bq[:], scale=1.0)
                # intra (negated): min(16dot - 16(1-ALPHA), 0); only the
                # diagonal of the one-hot accumulation below is used, so
                # computing it for every class costs nothing extra
                nc.vector.tensor_scalar(
                    qiv[:, :, C:2 * C], dot[:], float(ESC * (1.0 - ALPHA)),
                    0.0, ALU.subtract, ALU.min)
                qis.append(qi)
                if it >= DELAY:
                    emit_acc(it - DELAY)
            for i in range(IT - DELAY, IT):
                emit_acc(i)

            res = small.tile([C, 2 * C], F32)
            nc.vector.tensor_copy(res[:], ps_st[:])
            nc.sync.dma_start(out.ap()[:, :], res[:])

    nc.compile()
    return nc


def _prep(embeddings: np.ndarray, labels: np.ndarray):
    import ml_dtypes
    embf = np.asarray(embeddings, dtype=np.float32)
    lab = np.asarray(labels).astype(np.int64)
    nrm = np.maximum(np.sqrt((embf * embf).sum(1, keepdims=True)), 1e-8)
    e16 = (embf * (ESC / nrm)).astype(ml_dtypes.float8_e4m3)

    # host stage: class centroids (O(N*D) reduction) and the pair mask
    oh = np.zeros((N, C), np.float32)
    oh[np.arange(N), lab] = 1.0
    cnt = np.bincount(lab, minlength=C).astype(np.float32)
    sums = oh.T @ embf                                  # [C, D]
    cent = sums / np.maximum(cnt, 1.0)[:, None]
    chat = cent / np.maximum(np.linalg.norm(cent, axis=1, keepdims=True), 1e-8)
    pd = 1.0 - chat @ chat.T
    upper = np.triu(np.ones((C, C), bool), 1)
    present = cnt > 0
    pm = (upper & (pd <= BETA) & present[:, None] & present[None, :]
          ).astype(np.float64)
    pmsym = pm + pm.T

    chTl = np.ascontiguousarray(
        chat.T.reshape(KCH, P, C).transpose(1, 0, 2).reshape(P, KCH * C)
    ).astype(ml_dtypes.bfloat16)
    oh_b = oh.astype(ml_dtypes.bfloat16)

    in_maps = []
    for i in range(CORES):
        sl = slice(i * NL, (i + 1) * NL)
        e8s = e16[sl]
        e8Tl = np.ascontiguousarray(
            e8s.reshape(T, P, KCH, P).transpose(3, 0, 2, 1).reshape(P, T * 4 * P))
        ohp = np.ascontiguousarray(
            oh_b[sl].reshape(T, P, C).transpose(1, 0, 2).reshape(P, T * C))
        in_maps.append({"e8T": e8Tl, "ohb": ohp, "chTin": chTl})
    return in_maps, (cnt, pmsym)


def _finish(results, host_state):
    cnt, pmsym = host_state
    st = np.zeros((C, 2 * C), np.float64)
    for r in results:
        st += r["out"].astype(np.float64)
    st /= ESC
    STg, tg = st[:, :C], -np.diag(st[:, C:2 * C])
    deg = pmsym.sum(1)
    intra = float((deg * tg).sum())
    inter = float((pmsym * STg).sum())
    count = float((deg * cnt.astype(np.float64)).sum())
    num_pairs = pmsym.sum() / 2.0
    if num_pairs <= 0:
        return np.float32(0.0)
    return np.float32((intra + inter) / max(count, 1.0))


def kernel(embeddings: np.ndarray, labels: np.ndarray) -> np.ndarray:
    if "nc" not in _CACHE:
        _CACHE["nc"] = _build()
    nc = _CACHE["nc"]
    in_maps, host_state = _prep(embeddings, labels)
    res = run_bass_kernel_spmd(nc, in_maps, core_ids=list(range(CORES)))
    return _finish(res.results, host_state)


# revision 23
# speedup vs baseline: 1.1687x; 1.1687x over previous
"""AugmentedTripletLoss on 8 TRN2 NeuronCores — data-parallel Bass kernel.

v4 design (data-parallel over batch, 16384 samples/core):
  The O(N*C*D) work — all centroid-to-sample cosine distances, their
  relu thresholds, and the per-class masked reductions — runs on the
  8 cores. The tiny O(N*D) prep (normalization, fp8 quantization,
  class-centroid means, the [16,16] pair mask) and the final [16,17]
  gather-sum over the 8 local partials run on the host, so the device
  program has no collective (measured: the first collective in a NEFF
  pays a ~58us ncfw entry-barrier floor plus ~20us per AllReduce —
  about 45% of the v3 kernel span).

  Device per 512-sample iteration (32 iterations/core):
    16 accumulating matmuls (eT d-chunk stationary fp8, chT moving
    bf16) -> 16*dot in PSUM [128, 4x16];
    one ACT relu -> inter terms (strided into qi[., s, 0:16]);
    DVE masked negate-mult + grouped row-reduce -> own-class dots;
    DVE add+max -> intra terms qi[., s, 16];
    4 one-hot matmuls accumulate S^T ++ intra sums into ps_st [16,17].
  eT streams tile-major from HBM through a 10-deep 256KB chunk pool,
  rotating across the sync/scalar/vector HWDGE queues, so transfers
  overlap each other and the compute trails the stream by one chunk.
"""

import sys

sys.path.insert(0, "/opt/trn_rl_repo")

import numpy as np

import concourse.bass as bass
import concourse.bacc as bacc
import concourse.tile as tile
import concourse.mybir as mybir
from concourse.bass_utils import run_bass_kernel_spmd

ALPHA = 0.1
BETA = 1.1
C = 16
N = 131072
D = 512
CORES = 8
NL = N // CORES  # 16384 samples per core
P = 128
T = NL // P      # 128 sample tiles of 128 per core
B4 = 8           # sample tiles per device iteration
IT = T // B4     # 16 iterations
KCH = D // P     # 4 contraction chunks of 128
ESC = 16.0       # fp8 storage scale for ehat

F32 = mybir.dt.float32
BF16 = mybir.dt.bfloat16
FP8 = mybir.dt.float8e4
ALU = mybir.AluOpType
ACTF = mybir.ActivationFunctionType

_CACHE = {}


def _build():
    nc = bacc.Bacc("TRN2", target_bir_lowering=False, debug=False, num_devices=CORES)

    e8T = nc.dram_tensor("e8T", [P, T * 4 * P], FP8, kind="ExternalInput")
    ohb = nc.dram_tensor("ohb", [P, T * C], BF16, kind="ExternalInput")
    chTin = nc.dram_tensor("chTin", [P, KCH * C], BF16, kind="ExternalInput")
    out = nc.dram_tensor("out", [C, 2 * C], F32, kind="ExternalOutput")

    with tile.TileContext(nc) as tc:
        with (
            tc.tile_pool(name="pers", bufs=1) as pers,
            tc.tile_pool(name="work", bufs=6) as work,
            tc.tile_pool(name="ld", bufs=16) as ld,
            tc.tile_pool(name="small", bufs=1) as small,
            tc.tile_pool(name="psacc", bufs=1, space="PSUM") as psacc,
            tc.tile_pool(name="pstr", bufs=4, space="PSUM") as pstr,
        ):
            # ---- persistent SBUF state ----
            oh_sb = pers.tile([P, T * C], BF16)  # one-hot
            chT = pers.tile([P, KCH * C], BF16)  # d-major normalized centroids

            nc.sync.dma_start(chT[:], chTin[:, :])
            nc.gpsimd.dma_start(oh_sb[:], ohb[:, :])

            # the full 8MB input is SBUF-resident: issue every chunk DMA
            # upfront on the sync+scalar HWDGE queues before any compute
            # op enters those queues (gpsimd/SWDGE is avoided for bulk
            # streaming — the framework's mid-kernel DRAIN blocks it for
            # ~10us)
            HW_ = B4 * KCH * P // 2  # half-iteration chunk: 2048 cols (256KB)
            chunks = []
            for j in range(2 * IT):
                ech = ld.tile([P, HW_], FP8, tag=f"ech{j % 2}")
                eng = nc.sync if j % 2 == 0 else nc.scalar
                eng.dma_start(ech[:], e8T[:, j * HW_:(j + 1) * HW_])
                chunks.append(ech)
            ps_st = psacc.tile([C, 2 * C], F32)

            # the per-iteration one-hot accumulation matmuls are deferred
            # DELAY iterations: the PE queue is in-order, so an acc matmul
            # waiting on iteration i's ACT/DVE output must not sit in
            # front of iteration i+1's dot matmuls
            DELAY = 2
            qis = []

            def emit_acc(i):
                qi_i = qis[i]
                for s in range(B4):
                    t = B4 * i + s
                    nc.tensor.matmul(ps_st[:],
                                     oh_sb[:, t * C:(t + 1) * C],
                                     qi_i[:, s * 2 * C:(s + 1) * 2 * C],
                                     start=(t == 0), stop=(t == T - 1))

            for it in range(IT):
                ech0 = chunks[2 * it]
                ech1 = chunks[2 * it + 1]
                dot = pstr.tile([P, B4 * C], F32, tag="tp")
                for s in range(B4):
                    ech = ech0 if s < B4 // 2 else ech1
                    sl = s % (B4 // 2)
                    for k in range(KCH):
                        c0 = (sl * KCH + k) * P
                        nc.tensor.matmul(
                            dot[:, s * C:(s + 1) * C],
                            ech[:, c0:c0 + P],
                            chT[:, k * C:(k + 1) * C],
                            start=(k == 0), stop=(k == KCH - 1))
                qi = work.tile([P, B4 * 2 * C], BF16)
                qiv = qi[:].rearrange("p (s c) -> p s c", c=2 * C)
                # both thresholds on DVE so the scalar engine stays a pure
                # DMA-dispatch queue (an ACT in its queue would block the
                # chunk-DMA dispatches behind it)
                # inter: 16*relu(dot/16 + (BETA-1)) = max(16dot + 16(B-1), 0)
                nc.vector.tensor_scalar(
                    qiv[:, :, 0:C], dot[:], float(ESC * (BETA - 1.0)),
                    0.0, ALU.add, ALU.max)
                # intra (negated): min(16dot - 16(1-ALPHA), 0); only the
                # diagonal of the one-hot accumulation below is used, so
                # computing it for every class costs nothing extra
                nc.vector.tensor_scalar(
                    qiv[:, :, C:2 * C], dot[:], float(ESC * (1.0 - ALPHA)),
                    0.0, ALU.subtract, ALU.min)
                qis.append(qi)
                if it >= DELAY:
                    emit_acc(it - DELAY)
            for i in range(IT - DELAY, IT):
                emit_acc(i)

            res = small.tile([C, 2 * C], F32)
            nc.vector.tensor_copy(res[:], ps_st[:])
            nc.sync.dma_start(out.ap()[:, :], res[:])

    nc.compile()
    return nc


def _prep(embeddings: np.ndarray, labels: np.ndarray):
    import ml_dtypes
    embf = np.asarray(embeddings, dtype=np.float32)
    lab = np.asarray(labels).astype(np.int64)
    nrm = np.maximum(np.sqrt((embf * embf).sum(1, keepdims=True)), 1e-8)
    e16 = (embf * (ESC / nrm)).astype(ml_dtypes.float8_e4m3)

    # host stage: class centroids (O(N*D) reduction) and the pair mask
    oh = np.zeros((N, C), np.float32)
    oh[np.arange(N), lab] = 1.0
    cnt = np.bincount(lab, minlength=C).astype(np.float32)
    sums = oh.T @ embf                                  # [C, D]
    cent = sums / np.maximum(cnt, 1.0)[:, None]
    chat = cent / np.maximum(np.linalg.norm(cent, axis=1, keepdims=True), 1e-8)
    pd = 1.0 - chat @ chat.T
    upper = np.triu(np.ones((C, C), bool), 1)
    present = cnt > 0
    pm = (upper & (pd <= BETA) & present[:, None] & present[None, :]
          ).astype(np.float64)
    pmsym = pm + pm.T

    chTl = np.ascontiguousarray(
        chat.T.reshape(KCH, P, C).transpose(1, 0, 2).reshape(P, KCH * C)
    ).astype(ml_dtypes.bfloat16)
    oh_b = oh.astype(ml_dtypes.bfloat16)

    in_maps = []
    for i in range(CORES):
        sl = slice(i * NL, (i + 1) * NL)
        e8s = e16[sl]
        e8Tl = np.ascontiguousarray(
            e8s.reshape(T, P, KCH, P).transpose(3, 0, 2, 1).reshape(P, T * 4 * P))
        ohp = np.ascontiguousarray(
            oh_b[sl].reshape(T, P, C).transpose(1, 0, 2).reshape(P, T * C))
        in_maps.append({"e8T": e8Tl, "ohb": ohp, "chTin": chTl})
    return in_maps, (cnt, pmsym)


def _finish(results, host_state):
    cnt, pmsym = host_state
    st = np.zeros((C, 2 * C), np.float64)
    for r in results:
        st += r["out"].astype(np.float64)
    st /= ESC
    STg, tg = st[:, :C], -np.diag(st[:, C:2 * C])
    deg = pmsym.sum(1)
    intra = float((deg * tg).sum())
    inter = float((pmsym * STg).sum())
    count = float((deg * cnt.astype(np.float64)).sum())
    num_pairs = pmsym.sum() / 2.0
    if num_pairs <= 0:
        return np.float32(0.0)
    return np.float32((intra + inter) / max(count, 1.0))


def kernel(embeddings: np.ndarray, labels: np.ndarray) -> np.ndarray:
    if "nc" not in _CACHE:
        _CACHE["nc"] = _build()
    nc = _CACHE["nc"]
    in_maps, host_state = _prep(embeddings, labels)
    res = run_bass_kernel_spmd(nc, in_maps, core_ids=list(range(CORES)))
    return _finish(res.results, host_state)
